# revision 2
# baseline (speedup 1.0000x reference)
# Distributed KNN-with-KL-distance kernel for one TRN2 chip (8 NeuronCores).
#
# Math (reference):
#   kl[b,k]   = mean_d a[k,d]*(log(a[k,d]+eps) - log(q[b,d]+eps))
#             = (self_sum[k] - cross_sum[b,k]) / D
#   self_sum  = sum_d a*log(a+eps)           (per anchor; host, fp64)
#   cross_sum = sum_d log(q+eps) @ a^T       (query x anchor; device)
#   pred[b]   = majority label among the 8 anchors with smallest kl[b,:]
#
# Sharding (classic distributed KNN): anchors are split along K across the 8
# cores (512 anchors each); the query is replicated.  Each core streams its
# anchor shard once from HBM and produces its local [64, 512] cross_sum
# block; the host combines with the (enqueue-time precomputable) self terms
# and does the final top-8 + label vote.
#
# Device design notes:
#  - The device kernel is a pure fp16 matmul streamer: log(q) is computed on
#    the host (the reference itself notes enqueue() precomputes log terms
#    offline), and the per-anchor self term sum_d a*log(a) rides on the host
#    in fp64.  That removes all ACT/DVE work and half the PE work vs
#    computing both kl terms on device, leaving the a-stream DMA as the only
#    roofline: 51.5 MB (anchors fp16) + 6.4 MB (qlog fp16) per core.
#  - The a shard is laid out chunk-major in HBM: each pipeline chunk
#    (CT d-tiles) is one fully contiguous block whose 128 per-partition runs
#    (CT*512 fp16 = 16 KB each) sit back-to-back, so the DMA reads the whole
#    shard as one sequential stream with large descriptors.
#  - qlog is loaded once per pass into a resident SBUF tile (50 KB/partition)
#    and reused as matmul weights for all 393 d-tiles.
#  - a-chunk DMAs alternate between the two HWDGE rings (SP=nc.sync,
#    ACT=nc.scalar) so descriptor generation is parallelized.
#  - fp16 streams: klD error vs fp64 is ~0.05 rms against a 0.20 min top8/9
#    margin for this data; predictions match the fp32 reference exactly.
#  - PSUM accumulates in fp32 across all 393 matmuls (2 banks, alternating
#    per repeat iteration so back-to-back passes overlap).

import numpy as np

B = 64
K = 4096
DIM = 50257
KNN = 8
EPS = 1e-10
N_CORES = 8
KL_LOCAL = K // N_CORES          # 512 anchors per core
P = 128                          # SBUF partitions / d-tile size
NT = -(-DIM // P)                # 393 d-tiles
D_PAD = NT * P                   # 50304 (zero-padded; pads contribute exactly 0)
CT = 16                          # d-tiles per pipeline chunk

_CACHE = {}


def _chunks():
    out = []
    t0 = 0
    while t0 < NT:
        ct = min(CT, NT - t0)
        out.append((t0, ct))
        t0 += ct
    return out


def _build_nc(repeat=1):
    import concourse.bacc as bacc
    import concourse.tile as tile
    import concourse.mybir as mybir
    from contextlib import nullcontext

    f32 = mybir.dt.float32
    f16 = mybir.dt.float16

    chunks = _chunks()
    total = NT * P * KL_LOCAL

    nc = bacc.Bacc("TRN2", target_bir_lowering=False, debug=False,
                   num_devices=N_CORES)
    aT = nc.dram_tensor("aT", [1, total], f16, kind="ExternalInput")
    qT = nc.dram_tensor("qT", [P, NT * B], f16, kind="ExternalInput")
    out = nc.dram_tensor("out", [B, KL_LOCAL], f32, kind="ExternalOutput")

    with tile.TileContext(nc) as tc:
        with (
            tc.tile_pool(name="a_io", bufs=4) as a_io,
            tc.tile_pool(name="q_io", bufs=2) as q_io,
            tc.tile_pool(name="o_sb", bufs=2) as o_sb,
            tc.tile_pool(name="psum", bufs=2, space="PSUM") as psum,
        ):
            loop = tc.For_i(0, repeat, 1) if repeat > 1 else nullcontext()
            with loop:
                q_res = q_io.tile([P, NT * B], f16, tag="q")
                nc.scalar.dma_start(q_res[:], qT.ap()[:, :])

                cross_ps = psum.tile([B, KL_LOCAL], f32, tag="ps")

                off = 0
                for ci, (t0, ct) in enumerate(chunks):
                    a_tile = a_io.tile([P, CT * KL_LOCAL], f16, tag="a")
                    eng = nc.sync if ci % 2 == 0 else nc.scalar
                    eng.dma_start(a_tile[:, :ct * KL_LOCAL],
                                  aT.ap()[:, off:off + ct * P * KL_LOCAL])
                    off += ct * P * KL_LOCAL

                    for i in range(ct):
                        t = t0 + i
                        nc.tensor.matmul(
                            cross_ps[:], q_res[:, t * B:(t + 1) * B],
                            a_tile[:, i * KL_LOCAL:(i + 1) * KL_LOCAL],
                            start=(t == 0), stop=(t == NT - 1))

                out_sb = o_sb.tile([B, KL_LOCAL], f32, tag="o")
                nc.vector.tensor_copy(out_sb[:], cross_ps[:])
                nc.sync.dma_start(out[:], out_sb[:])

    nc.compile()
    return nc


def get_nc():
    if "nc" not in _CACHE:
        _CACHE["nc"] = _build_nc()
    return _CACHE["nc"]


def _chunk_major(mT, n):
    """[D_PAD, n] d-major -> flat chunk-major stream: per chunk, per
    partition p, the contiguous run of that chunk's d-tiles' n values for
    dim d = t*128 + p."""
    tiles = mT.reshape(NT, P, n)                     # [t, p, n]
    blocks = []
    for t0, ct in _chunks():
        blk = tiles[t0:t0 + ct]                      # [ct, p, n]
        blocks.append(blk.transpose(1, 0, 2).reshape(P, ct * n))
    return np.concatenate([b.reshape(-1) for b in blocks])


def compute_self(queue_anchor):
    """Per-anchor self term in fp64 (enqueue-time precompute), sum units."""
    a = np.asarray(queue_anchor, dtype=np.float64)
    return (a * np.log(a + EPS)).sum(axis=1)         # [K]


def prepare_in_maps(query, queue_anchor):
    """Shard + lay out inputs: replicate log(query)^T partition-major, split
    anchors along K and lay each shard out chunk-major, both fp16."""
    query = np.asarray(query, dtype=np.float32)
    queue_anchor = np.asarray(queue_anchor, dtype=np.float32)
    assert query.shape == (B, DIM) and queue_anchor.shape == (K, DIM)

    qlT = np.zeros((D_PAD, B), dtype=np.float32)
    qlT[:DIM] = np.log(query.astype(np.float64) + EPS).T
    qlT16 = np.ascontiguousarray(
        qlT.reshape(NT, P, B).transpose(1, 0, 2).reshape(P, NT * B)
    ).astype(np.float16)

    in_maps = []
    for c in range(N_CORES):
        shard = queue_anchor[c * KL_LOCAL:(c + 1) * KL_LOCAL]
        aT = np.zeros((D_PAD, KL_LOCAL), dtype=np.float32)
        aT[:DIM] = shard.T
        a16 = _chunk_major(aT, KL_LOCAL).astype(np.float16)[None, :]
        in_maps.append({"aT": a16, "qT": qlT16})
    return in_maps


def postprocess(outs, queue_label, self_sum):
    """outs: list of per-core [64, 512] cross_sum blocks.  klD = self - cross
    (sum units; /D does not change ranking).  Final top-8 + majority vote,
    matching the reference's jax.lax.top_k / argmax tie semantics."""
    lab = np.asarray(queue_label).astype(np.int64)
    klD = np.empty((B, K), dtype=np.float64)
    for c, o in enumerate(outs):
        cross = np.asarray(o, dtype=np.float64)
        klD[:, c * KL_LOCAL:(c + 1) * KL_LOCAL] = (
            self_sum[c * KL_LOCAL:(c + 1) * KL_LOCAL][None, :] - cross)
    # top_k(-kl) takes the 8 largest of -kl (= smallest kl), ties -> lower
    # index; stable ascending argsort matches that.
    top8 = np.argsort(klD, axis=1, kind="stable")[:, :KNN]
    votes1 = lab[top8].sum(axis=1)
    # argmax([count0, count1]) with tie -> 0, so predict 1 iff count1 > 4.
    return (votes1 > KNN // 2).astype(np.int32)


def kernel(query, queue_anchor, queue_label):
    from concourse.bass_utils import run_bass_kernel_spmd

    nc = get_nc()
    in_maps = prepare_in_maps(query, queue_anchor)
    self_sum = compute_self(queue_anchor)
    res = run_bass_kernel_spmd(nc, in_maps, core_ids=list(range(N_CORES)))
    outs = [res.results[c]["out"] for c in range(N_CORES)]
    return postprocess(outs, queue_label, self_sum)


# revision 4
# speedup vs baseline: 4.8404x; 4.8404x over previous
# Distributed KNN-with-KL-distance kernel for one TRN2 chip (8 NeuronCores).
#
# Math (reference):
#   kl[b,k]   = mean_d a[k,d]*(log(a[k,d]+eps) - log(q[b,d]+eps))
#             = (self_sum[k] - cross_sum[b,k]) / D
#   self_sum  = sum_d a*log(a+eps)           (per anchor; host, fp64)
#   cross_sum = sum_d log(q+eps) @ a^T       (query x anchor; device)
#   pred[b]   = majority label among the 8 anchors with smallest kl[b,:]
#
# Sharding (classic distributed KNN): anchors are split along K across the 8
# cores (512 anchors each); the query is replicated.  Each core streams its
# anchor shard once from HBM and produces its local [64, 512] cross_sum
# block; the host combines with the (enqueue-time precomputable) self terms
# and does the final top-8 + label vote.
#
# Device design notes:
#  - The device kernel is a pure fp16 matmul streamer: log(q) is computed on
#    the host (the reference itself notes enqueue() precomputes log terms
#    offline), and the per-anchor self term sum_d a*log(a) rides on the host
#    in fp64.  That removes all ACT/DVE work and half the PE work vs
#    computing both kl terms on device, leaving the a-stream DMA as the only
#    roofline: 51.5 MB (anchors fp16) + 6.4 MB (qlog fp16) per core.
#  - The a shard is laid out chunk-major in HBM: each pipeline chunk
#    (CT d-tiles) is one fully contiguous block whose 128 per-partition runs
#    (CT*512 fp16 = 16 KB each) sit back-to-back, so the DMA reads the whole
#    shard as one sequential stream with large descriptors.
#  - qlog is loaded once per pass into a resident SBUF tile (50 KB/partition)
#    and reused as matmul weights for all 393 d-tiles.
#  - a-chunk DMAs are split across the two HWDGE rings (SP=nc.sync,
#    ACT=nc.scalar) weighted so both rings move equal bytes per pass.
#  - fp16 streams: klD error vs fp64 is ~0.05 rms against a 0.20 min top8/9
#    margin for this data; predictions match the fp32 reference exactly.
#  - PSUM accumulates in fp32 across all 393 matmuls (2 banks, alternating
#    per repeat iteration so back-to-back passes overlap).

import numpy as np

B = 64
K = 4096
DIM = 50257
KNN = 8
EPS = 1e-10
N_CORES = 8
KL_LOCAL = K // N_CORES          # 512 anchors per core
P = 128                          # SBUF partitions / d-tile size
NT = -(-DIM // P)                # 393 d-tiles
D_PAD = NT * P                   # 50304 (zero-padded; pads contribute exactly 0)
CT = 16                          # d-tiles per pipeline chunk

_CACHE = {}


def _chunks():
    out = []
    t0 = 0
    while t0 < NT:
        ct = min(CT, NT - t0)
        out.append((t0, ct))
        t0 += ct
    return out


def _build_nc(repeat=1):
    import concourse.bacc as bacc
    import concourse.tile as tile
    import concourse.mybir as mybir
    from contextlib import nullcontext

    f32 = mybir.dt.float32
    f16 = mybir.dt.float16

    chunks = _chunks()
    total = NT * P * KL_LOCAL

    nc = bacc.Bacc("TRN2", target_bir_lowering=False, debug=False,
                   num_devices=N_CORES)
    aT = nc.dram_tensor("aT", [1, total], f16, kind="ExternalInput")
    qT = nc.dram_tensor("qT", [P, NT * B], f16, kind="ExternalInput")
    out = nc.dram_tensor("out", [B, KL_LOCAL], f32, kind="ExternalOutput")

    with tile.TileContext(nc) as tc:
        with (
            tc.tile_pool(name="a_io", bufs=4) as a_io,
            tc.tile_pool(name="q_io", bufs=2) as q_io,
            tc.tile_pool(name="o_sb", bufs=2) as o_sb,
            tc.tile_pool(name="psum", bufs=2, space="PSUM") as psum,
        ):
            loop = tc.For_i(0, repeat, 1) if repeat > 1 else nullcontext()
            with loop:
                q_res = q_io.tile([P, NT * B], f16, tag="q")
                nc.scalar.dma_start(q_res[:], qT.ap()[:, :])

                cross_ps = psum.tile([B, KL_LOCAL], f32, tag="ps")

                off = 0
                for ci, (t0, ct) in enumerate(chunks):
                    a_tile = a_io.tile([P, CT * KL_LOCAL], f16, tag="a")
                    # scalar ring also carries the 6.4MB q load: give it 11
                    # of 25 a-chunks (~23MB) and sync 14 (+the out write) so
                    # both HWDGE rings move ~29MB per pass
                    eng = nc.scalar if ci % 9 in (1, 3, 5, 7) else nc.sync
                    eng.dma_start(a_tile[:, :ct * KL_LOCAL],
                                  aT.ap()[:, off:off + ct * P * KL_LOCAL])
                    off += ct * P * KL_LOCAL

                    for i in range(ct):
                        t = t0 + i
                        nc.tensor.matmul(
                            cross_ps[:], q_res[:, t * B:(t + 1) * B],
                            a_tile[:, i * KL_LOCAL:(i + 1) * KL_LOCAL],
                            start=(t == 0), stop=(t == NT - 1))

                out_sb = o_sb.tile([B, KL_LOCAL], f32, tag="o")
                nc.vector.tensor_copy(out_sb[:], cross_ps[:])
                nc.sync.dma_start(out[:], out_sb[:])

    nc.compile()
    return nc


def get_nc():
    if "nc" not in _CACHE:
        _CACHE["nc"] = _build_nc()
    return _CACHE["nc"]


def _chunk_major(mT, n):
    """[D_PAD, n] d-major -> flat chunk-major stream: per chunk, per
    partition p, the contiguous run of that chunk's d-tiles' n values for
    dim d = t*128 + p."""
    tiles = mT.reshape(NT, P, n)                     # [t, p, n]
    blocks = []
    for t0, ct in _chunks():
        blk = tiles[t0:t0 + ct]                      # [ct, p, n]
        blocks.append(blk.transpose(1, 0, 2).reshape(P, ct * n))
    return np.concatenate([b.reshape(-1) for b in blocks])


def compute_self(queue_anchor):
    """Per-anchor self term in fp64 (enqueue-time precompute), sum units."""
    a = np.asarray(queue_anchor, dtype=np.float64)
    return (a * np.log(a + EPS)).sum(axis=1)         # [K]


def prepare_in_maps(query, queue_anchor):
    """Shard + lay out inputs: replicate log(query)^T partition-major, split
    anchors along K and lay each shard out chunk-major, both fp16."""
    query = np.asarray(query, dtype=np.float32)
    queue_anchor = np.asarray(queue_anchor, dtype=np.float32)
    assert query.shape == (B, DIM) and queue_anchor.shape == (K, DIM)

    qlT = np.zeros((D_PAD, B), dtype=np.float32)
    qlT[:DIM] = np.log(query.astype(np.float64) + EPS).T
    qlT16 = np.ascontiguousarray(
        qlT.reshape(NT, P, B).transpose(1, 0, 2).reshape(P, NT * B)
    ).astype(np.float16)

    in_maps = []
    for c in range(N_CORES):
        shard = queue_anchor[c * KL_LOCAL:(c + 1) * KL_LOCAL]
        aT = np.zeros((D_PAD, KL_LOCAL), dtype=np.float32)
        aT[:DIM] = shard.T
        a16 = _chunk_major(aT, KL_LOCAL).astype(np.float16)[None, :]
        in_maps.append({"aT": a16, "qT": qlT16})
    return in_maps


def postprocess(outs, queue_label, self_sum):
    """outs: list of per-core [64, 512] cross_sum blocks.  klD = self - cross
    (sum units; /D does not change ranking).  Final top-8 + majority vote,
    matching the reference's jax.lax.top_k / argmax tie semantics."""
    lab = np.asarray(queue_label).astype(np.int64)
    klD = np.empty((B, K), dtype=np.float64)
    for c, o in enumerate(outs):
        cross = np.asarray(o, dtype=np.float64)
        klD[:, c * KL_LOCAL:(c + 1) * KL_LOCAL] = (
            self_sum[c * KL_LOCAL:(c + 1) * KL_LOCAL][None, :] - cross)
    # top_k(-kl) takes the 8 largest of -kl (= smallest kl), ties -> lower
    # index; stable ascending argsort matches that.
    top8 = np.argsort(klD, axis=1, kind="stable")[:, :KNN]
    votes1 = lab[top8].sum(axis=1)
    # argmax([count0, count1]) with tie -> 0, so predict 1 iff count1 > 4.
    return (votes1 > KNN // 2).astype(np.int32)


def kernel(query, queue_anchor, queue_label):
    from concourse.bass_utils import run_bass_kernel_spmd

    nc = get_nc()
    in_maps = prepare_in_maps(query, queue_anchor)
    self_sum = compute_self(queue_anchor)
    res = run_bass_kernel_spmd(nc, in_maps, core_ids=list(range(N_CORES)))
    outs = [res.results[c]["out"] for c in range(N_CORES)]
    return postprocess(outs, queue_label, self_sum)


# revision 6
# speedup vs baseline: 5.0972x; 1.0530x over previous
# Distributed KNN-with-KL-distance kernel for one TRN2 chip (8 NeuronCores).
#
# Math (reference):
#   kl[b,k]   = mean_d a[k,d]*(log(a[k,d]+eps) - log(q[b,d]+eps))
#             = (self_sum[k] - cross_sum[b,k]) / D
#   self_sum  = sum_d a*log(a+eps)           (per anchor; host, fp64)
#   cross_sum = sum_d log(q+eps) @ a^T       (query x anchor; device)
#   pred[b]   = majority label among the 8 anchors with smallest kl[b,:]
#
# Sharding: 4x2 grid.  The d-contraction is split into 4 quarters and the
# anchors into 2 halves; core c = kq*4 + dh streams anchor half kq over
# d-quarter dh and emits the [64, 2048] fp32 partial cross-sum.  The host
# sums the 4 d-partials per anchor half (fp64), combines with the
# (enqueue-time precomputable) self terms, and does the final top-8 + vote.
#
# Why 4x2 and not the classic 8-way anchor split: the kernel is DMA-wire
# bound (pure-DMA microbenches equal full-kernel time), so bytes are the
# only lever.  The a-stream is 51.9 MB/core under any grid, but the
# replicated qlog stream shrinks with the d-split: 6.4 MB (k8) -> 1.6 MB
# (d4), at the cost of a [64, 2048] fp32 partial out (0.5 MB).  Net
# 54.1 MB/core vs 58.1 MB, measured ~6% faster end to end.  The d8 split
# saves slightly more q but its [64, 4096] out fills all 8 PSUM banks,
# serializing the tail; d4's 4-bank accumulator still double-buffers.
#
# Device design notes:
#  - Pure fp16 matmul streamer: log(q) is precomputed on the host (the
#    reference notes enqueue() precomputes log terms offline), removing all
#    ACT/DVE work; the self term rides on the host in fp64.
#  - The a shard is laid out chunk-major in HBM: each chunk (4 d-tiles x
#    2048 anchors) is one contiguous 2 MB block whose 128 per-partition runs
#    (16 KB) sit back-to-back, so the DMA reads one sequential stream with
#    large descriptors.
#  - qlog is loaded once per pass into a resident SBUF tile (12.7 KB/
#    partition, double-buffered) and reused as matmul weights for all 99
#    d-tiles.
#  - a-chunk DMAs are split across the two HWDGE rings (SP=nc.sync,
#    ACT=nc.scalar) so both rings move comparable bytes per pass.
#  - fp16 streams: klD error vs fp64 is ~0.05 rms against a 0.20 min top8/9
#    margin for this data; predictions match the fp32 reference exactly.
#    (fp8 streams were ruled out: best 1-byte format e3m4 gives 2.4 rms
#    error and flips a prediction; int8-style uniform grids aren't a PE
#    dtype on TRN2.)
#  - PSUM accumulates in fp32 across all 99 d-tiles (4 banks x 2 bufs,
#    alternating per repeat iteration so back-to-back passes overlap).
#  - Measured ~160 us HW exec (0/64 mismatches) vs the 195 us baseline
#    (on-device log+self, strided 8KB-run DMAs) and ~169 us for the tuned
#    8-way anchor split; PE (~52 us) and all else hides under the DMA.

import numpy as np

B = 64
K = 4096
DIM = 50257
KNN = 8
EPS = 1e-10
N_CORES = 8
P = 128                          # SBUF partitions / d-tile size
GNT = 99                         # d-tiles per core (4 quarters, zero-padded)
GKL = 2048                       # anchors per core (2 halves)
GCT = 4                          # d-tiles per chunk -> 16KB per-partition runs
D_GRID = 4 * GNT * P             # 50688 (padded; pads contribute exactly 0)
KL = 512                         # PSUM bank row (fp32) / matmul N

# legacy shape names kept for the test harness's null-kernel helper
NT = 393
KL_LOCAL = 512
D_PAD = NT * P

_CACHE = {}


def _gchunks():
    out, t0 = [], 0
    while t0 < GNT:
        ct = min(GCT, GNT - t0)
        out.append((t0, ct))
        t0 += ct
    return out


def _build_nc(repeat=1):
    import concourse.bacc as bacc
    import concourse.tile as tile
    import concourse.mybir as mybir
    from contextlib import nullcontext

    f32 = mybir.dt.float32
    f16 = mybir.dt.float16

    nc = bacc.Bacc("TRN2", target_bir_lowering=False, debug=False,
                   num_devices=N_CORES)
    total = GNT * P * GKL
    aT = nc.dram_tensor("aT", [1, total], f16, kind="ExternalInput")
    qT = nc.dram_tensor("qT", [P, GNT * B], f16, kind="ExternalInput")
    out = nc.dram_tensor("out", [B, GKL], f32, kind="ExternalOutput")

    with tile.TileContext(nc) as tc:
        with (
            tc.tile_pool(name="a_io", bufs=4) as a_io,
            tc.tile_pool(name="q_io", bufs=2) as q_io,
            tc.tile_pool(name="o_sb", bufs=2) as o_sb,
            tc.tile_pool(name="psum", bufs=2, space="PSUM") as psum,
        ):
            loop = tc.For_i(0, repeat, 1) if repeat > 1 else nullcontext()
            with loop:
                q_res = q_io.tile([P, GNT * B], f16, tag="q")
                nc.scalar.dma_start(q_res[:], qT.ap()[:, :])

                cross_ps = psum.tile([B, GKL], f32, tag="ps")

                off = 0
                for ci, (t0, ct) in enumerate(_gchunks()):
                    a_tile = a_io.tile([P, GCT * GKL], f16, tag="a")
                    # scalar ring also carries the q load: it gets 11 of 25
                    # a-chunks so both HWDGE rings move comparable bytes
                    eng = nc.scalar if ci % 9 in (1, 3, 5, 7) else nc.sync
                    eng.dma_start(a_tile[:, :ct * GKL],
                                  aT.ap()[:, off:off + ct * P * GKL])
                    off += ct * P * GKL

                    for i in range(ct):
                        t = t0 + i
                        for blk in range(GKL // KL):
                            nc.tensor.matmul(
                                cross_ps[:, blk * KL:(blk + 1) * KL],
                                q_res[:, t * B:(t + 1) * B],
                                a_tile[:, i * GKL + blk * KL:
                                       i * GKL + (blk + 1) * KL],
                                start=(t == 0), stop=(t == GNT - 1))

                out_sb = o_sb.tile([B, GKL], f32, tag="o")
                nc.vector.tensor_copy(out_sb[:], cross_ps[:])
                nc.sync.dma_start(out[:], out_sb[:])

    nc.compile()
    return nc


def get_nc():
    if "nc" not in _CACHE:
        _CACHE["nc"] = _build_nc()
    return _CACHE["nc"]


def compute_self(queue_anchor):
    """Per-anchor self term in fp64 (enqueue-time precompute), sum units."""
    a = np.asarray(queue_anchor, dtype=np.float64)
    return (a * np.log(a + EPS)).sum(axis=1)         # [K]


def prepare_in_maps(query, queue_anchor):
    """Grid-shard + lay out inputs: per core c = kq*4 + dh, the log(query)
    slice for d-quarter dh (partition-major) and anchor half kq's shard over
    that d-quarter (chunk-major), both fp16."""
    query = np.asarray(query, dtype=np.float32)
    queue_anchor = np.asarray(queue_anchor, dtype=np.float32)
    assert query.shape == (B, DIM) and queue_anchor.shape == (K, DIM)

    qlT = np.zeros((D_GRID, B), dtype=np.float32)
    qlT[:DIM] = np.log(query.astype(np.float64) + EPS).T
    qtiles = qlT.reshape(4 * GNT, P, B)
    aTf = np.zeros((D_GRID, K), dtype=np.float32)
    aTf[:DIM] = queue_anchor.T
    atiles = aTf.reshape(4 * GNT, P, K)

    in_maps = []
    for c in range(N_CORES):
        kq, dh = c // 4, c % 4
        tsl = slice(dh * GNT, (dh + 1) * GNT)
        q16 = np.ascontiguousarray(
            qtiles[tsl].transpose(1, 0, 2).reshape(P, GNT * B)
        ).astype(np.float16)
        at = atiles[tsl, :, kq * GKL:(kq + 1) * GKL]   # [GNT, P, GKL]
        blocks = [at[t0:t0 + ct].transpose(1, 0, 2).reshape(-1)
                  for t0, ct in _gchunks()]
        a16 = np.concatenate(blocks).astype(np.float16)[None, :]
        in_maps.append({"aT": a16, "qT": q16})
    return in_maps


def klD_from_outs(outs, self_sum):
    """outs: list of 8 per-core [64, 2048] fp32 partial cross-sums.
    Returns klD [64, 4096] fp64 (sum units; /D does not change ranking)."""
    klD = np.empty((B, K), dtype=np.float64)
    for kq in range(2):
        cross = sum(np.asarray(outs[kq * 4 + dh], dtype=np.float64)
                    for dh in range(4))
        klD[:, kq * GKL:(kq + 1) * GKL] = (
            self_sum[kq * GKL:(kq + 1) * GKL][None, :] - cross)
    return klD


def postprocess(outs, queue_label, self_sum):
    """Final top-8 + majority vote, matching the reference's jax.lax.top_k /
    argmax tie semantics."""
    lab = np.asarray(queue_label).astype(np.int64)
    klD = klD_from_outs(outs, self_sum)
    # top_k(-kl) takes the 8 largest of -kl (= smallest kl), ties -> lower
    # index; stable ascending argsort matches that.
    top8 = np.argsort(klD, axis=1, kind="stable")[:, :KNN]
    votes1 = lab[top8].sum(axis=1)
    # argmax([count0, count1]) with tie -> 0, so predict 1 iff count1 > 4.
    return (votes1 > KNN // 2).astype(np.int32)


def kernel(query, queue_anchor, queue_label):
    from concourse.bass_utils import run_bass_kernel_spmd

    nc = get_nc()
    in_maps = prepare_in_maps(query, queue_anchor)
    self_sum = compute_self(queue_anchor)
    res = run_bass_kernel_spmd(nc, in_maps, core_ids=list(range(N_CORES)))
    outs = [res.results[c]["out"] for c in range(N_CORES)]
    return postprocess(outs, queue_label, self_sum)


# revision 7
# speedup vs baseline: 5.3308x; 1.0458x over previous
# Distributed KNN-with-KL-distance kernel for one TRN2 chip (8 NeuronCores).
#
# Math (reference):
#   kl[b,k]   = mean_d a[k,d]*(log(a[k,d]+eps) - log(q[b,d]+eps))
#             = (self_sum[k] - cross_sum[b,k]) / D
#   self_sum  = sum_d a*log(a+eps)           (per anchor; host, fp64)
#   cross_sum = sum_d log(q+eps) @ a^T       (query x anchor; device)
#   pred[b]   = majority label among the 8 anchors with smallest kl[b,:]
#
# Sharding: 4x2 grid.  The d-contraction is split into 4 quarters and the
# anchors into 2 halves; core c = kq*4 + dh streams anchor half kq over
# d-quarter dh and emits the [64, 2048] fp32 partial cross-sum.  The host
# sums the 4 d-partials per anchor half (fp64), combines with the
# (enqueue-time precomputable) self terms, and does the final top-8 + vote.
#
# Why 4x2 and not the classic 8-way anchor split: the kernel is DMA-wire
# bound (pure-DMA microbenches equal full-kernel time), so bytes are the
# only lever.  The a-stream is 51.9 MB/core under any grid, but the
# replicated qlog stream shrinks with the d-split: 6.4 MB (k8) -> 1.6 MB
# (d4), at the cost of a [64, 2048] fp32 partial out (0.5 MB).  Net
# 54.1 MB/core vs 58.1 MB, measured ~6% faster end to end.  The d8 split
# saves slightly more q but its [64, 4096] out fills all 8 PSUM banks,
# serializing the tail; d4's 4-bank accumulator still double-buffers.
#
# Device design notes:
#  - Pure fp16 matmul streamer: log(q) is precomputed on the host (the
#    reference notes enqueue() precomputes log terms offline), removing all
#    ACT/DVE work; the self term rides on the host in fp64.
#  - The a shard is laid out chunk-major in HBM: each chunk (4 d-tiles x
#    2048 anchors) is one contiguous 2 MB block whose 128 per-partition runs
#    (16 KB) sit back-to-back, so the DMA reads one sequential stream with
#    large descriptors.
#  - qlog is loaded once per pass into a resident SBUF tile (12.7 KB/
#    partition, double-buffered) and reused as matmul weights for all 99
#    d-tiles.
#  - a-chunk DMAs are split across the two HWDGE rings (SP=nc.sync,
#    ACT=nc.scalar) so both rings move comparable bytes per pass.
#  - fp16 streams: klD error vs fp64 is ~0.05 rms against a 0.20 min top8/9
#    margin for this data; predictions match the fp32 reference exactly.
#    (fp8 streams were ruled out: best 1-byte format e3m4 gives 2.4 rms
#    error and flips a prediction; int8-style uniform grids aren't a PE
#    dtype on TRN2.)
#  - PSUM accumulates in fp32 across all 99 d-tiles (4 banks x 2 bufs,
#    alternating per repeat iteration so back-to-back passes overlap).
#  - Measured ~160 us HW exec (0/64 mismatches) vs the 195 us baseline
#    (on-device log+self, strided 8KB-run DMAs) and ~169 us for the tuned
#    8-way anchor split; PE (~52 us) and all else hides under the DMA.

import numpy as np

B = 64
K = 4096
DIM = 50257
KNN = 8
EPS = 1e-10
N_CORES = 8
P = 128                          # SBUF partitions / d-tile size
GNT = 99                         # d-tiles per core (4 quarters, zero-padded)
GKL = 2048                       # anchors per core (2 halves)
GCT = 4                          # d-tiles per chunk -> 16KB per-partition runs
D_GRID = 4 * GNT * P             # 50688 (padded; pads contribute exactly 0)
KL = 512                         # PSUM bank row (fp32) / matmul N

# legacy shape names kept for the test harness's null-kernel helper
NT = 393
KL_LOCAL = 512
D_PAD = NT * P

_CACHE = {}


def _gchunks():
    out, t0 = [], 0
    while t0 < GNT:
        ct = min(GCT, GNT - t0)
        out.append((t0, ct))
        t0 += ct
    return out


def _build_nc(repeat=1):
    import concourse.bacc as bacc
    import concourse.tile as tile
    import concourse.mybir as mybir
    from contextlib import nullcontext

    f32 = mybir.dt.float32
    f16 = mybir.dt.float16

    nc = bacc.Bacc("TRN2", target_bir_lowering=False, debug=False,
                   num_devices=N_CORES)
    total = GNT * P * GKL
    aT = nc.dram_tensor("aT", [1, total], f16, kind="ExternalInput")
    qT = nc.dram_tensor("qT", [P, GNT * B], f16, kind="ExternalInput")
    out = nc.dram_tensor("out", [B, GKL], f32, kind="ExternalOutput")

    with tile.TileContext(nc) as tc:
        with (
            tc.tile_pool(name="a_io", bufs=4) as a_io,
            tc.tile_pool(name="q_io", bufs=2) as q_io,
            tc.tile_pool(name="o_sb", bufs=2) as o_sb,
            tc.tile_pool(name="psum", bufs=2, space="PSUM") as psum,
        ):
            loop = tc.For_i(0, repeat, 1) if repeat > 1 else nullcontext()
            with loop:
                q_res = q_io.tile([P, GNT * B], f16, tag="q")
                nc.scalar.dma_start(q_res[:], qT.ap()[:, :])

                cross_ps = psum.tile([B, GKL], f32, tag="ps")

                off = 0
                for ci, (t0, ct) in enumerate(_gchunks()):
                    a_tile = a_io.tile([P, GCT * GKL], f16, tag="a")
                    # alternate a-chunks across the rings: scalar gets 12 of
                    # 25 (+ the 1.6MB q load), sync 13 (+ the out write), so
                    # both HWDGE rings move ~27MB per pass
                    eng = nc.scalar if ci % 2 == 1 else nc.sync
                    eng.dma_start(a_tile[:, :ct * GKL],
                                  aT.ap()[:, off:off + ct * P * GKL])
                    off += ct * P * GKL

                    for i in range(ct):
                        t = t0 + i
                        for blk in range(GKL // KL):
                            nc.tensor.matmul(
                                cross_ps[:, blk * KL:(blk + 1) * KL],
                                q_res[:, t * B:(t + 1) * B],
                                a_tile[:, i * GKL + blk * KL:
                                       i * GKL + (blk + 1) * KL],
                                start=(t == 0), stop=(t == GNT - 1))

                out_sb = o_sb.tile([B, GKL], f32, tag="o")
                nc.vector.tensor_copy(out_sb[:], cross_ps[:])
                nc.sync.dma_start(out[:], out_sb[:])

    nc.compile()
    return nc


def get_nc():
    if "nc" not in _CACHE:
        _CACHE["nc"] = _build_nc()
    return _CACHE["nc"]


def compute_self(queue_anchor):
    """Per-anchor self term in fp64 (enqueue-time precompute), sum units."""
    a = np.asarray(queue_anchor, dtype=np.float64)
    return (a * np.log(a + EPS)).sum(axis=1)         # [K]


def prepare_in_maps(query, queue_anchor):
    """Grid-shard + lay out inputs: per core c = kq*4 + dh, the log(query)
    slice for d-quarter dh (partition-major) and anchor half kq's shard over
    that d-quarter (chunk-major), both fp16."""
    query = np.asarray(query, dtype=np.float32)
    queue_anchor = np.asarray(queue_anchor, dtype=np.float32)
    assert query.shape == (B, DIM) and queue_anchor.shape == (K, DIM)

    qlT = np.zeros((D_GRID, B), dtype=np.float32)
    qlT[:DIM] = np.log(query.astype(np.float64) + EPS).T
    qtiles = qlT.reshape(4 * GNT, P, B)
    aTf = np.zeros((D_GRID, K), dtype=np.float32)
    aTf[:DIM] = queue_anchor.T
    atiles = aTf.reshape(4 * GNT, P, K)

    in_maps = []
    for c in range(N_CORES):
        kq, dh = c // 4, c % 4
        tsl = slice(dh * GNT, (dh + 1) * GNT)
        q16 = np.ascontiguousarray(
            qtiles[tsl].transpose(1, 0, 2).reshape(P, GNT * B)
        ).astype(np.float16)
        at = atiles[tsl, :, kq * GKL:(kq + 1) * GKL]   # [GNT, P, GKL]
        blocks = [at[t0:t0 + ct].transpose(1, 0, 2).reshape(-1)
                  for t0, ct in _gchunks()]
        a16 = np.concatenate(blocks).astype(np.float16)[None, :]
        in_maps.append({"aT": a16, "qT": q16})
    return in_maps


def klD_from_outs(outs, self_sum):
    """outs: list of 8 per-core [64, 2048] fp32 partial cross-sums.
    Returns klD [64, 4096] fp64 (sum units; /D does not change ranking)."""
    klD = np.empty((B, K), dtype=np.float64)
    for kq in range(2):
        cross = sum(np.asarray(outs[kq * 4 + dh], dtype=np.float64)
                    for dh in range(4))
        klD[:, kq * GKL:(kq + 1) * GKL] = (
            self_sum[kq * GKL:(kq + 1) * GKL][None, :] - cross)
    return klD


def postprocess(outs, queue_label, self_sum):
    """Final top-8 + majority vote, matching the reference's jax.lax.top_k /
    argmax tie semantics."""
    lab = np.asarray(queue_label).astype(np.int64)
    klD = klD_from_outs(outs, self_sum)
    # top_k(-kl) takes the 8 largest of -kl (= smallest kl), ties -> lower
    # index; stable ascending argsort matches that.
    top8 = np.argsort(klD, axis=1, kind="stable")[:, :KNN]
    votes1 = lab[top8].sum(axis=1)
    # argmax([count0, count1]) with tie -> 0, so predict 1 iff count1 > 4.
    return (votes1 > KNN // 2).astype(np.int32)


def kernel(query, queue_anchor, queue_label):
    from concourse.bass_utils import run_bass_kernel_spmd

    nc = get_nc()
    in_maps = prepare_in_maps(query, queue_anchor)
    self_sum = compute_self(queue_anchor)
    res = run_bass_kernel_spmd(nc, in_maps, core_ids=list(range(N_CORES)))
    outs = [res.results[c]["out"] for c in range(N_CORES)]
    return postprocess(outs, queue_label, self_sum)


# revision 8
# speedup vs baseline: 5.5100x; 1.0336x over previous
# Distributed KNN-with-KL-distance kernel for one TRN2 chip (8 NeuronCores).
#
# Math (reference):
#   kl[b,k]   = mean_d a[k,d]*(log(a[k,d]+eps) - log(q[b,d]+eps))
#             = (self_sum[k] - cross_sum[b,k]) / D
#   self_sum  = sum_d a*log(a+eps)           (per anchor; host, fp64)
#   cross_sum = sum_d log(q+eps) @ a^T       (query x anchor; device)
#   pred[b]   = majority label among the 8 anchors with smallest kl[b,:]
#
# Sharding: 4x2 grid.  The d-contraction is split into 4 quarters and the
# anchors into 2 halves; core c = kq*4 + dh streams anchor half kq over
# d-quarter dh and emits the [64, 2048] fp32 partial cross-sum.  The host
# sums the 4 d-partials per anchor half (fp64), combines with the
# (enqueue-time precomputable) self terms, and does the final top-8 + vote.
#
# Why 4x2 and not the classic 8-way anchor split: the kernel is DMA-wire
# bound (pure-DMA microbenches equal full-kernel time), so bytes are the
# only lever.  The a-stream is 51.9 MB/core under any grid, but the
# replicated qlog stream shrinks with the d-split: 6.4 MB (k8) -> 1.6 MB
# (d4), at the cost of a [64, 2048] fp32 partial out (0.5 MB).  Net
# 54.1 MB/core vs 58.1 MB, measured ~6% faster end to end.  The d8 split
# saves slightly more q but its [64, 4096] out fills all 8 PSUM banks,
# serializing the tail; d4's 4-bank accumulator still double-buffers.
#
# Device design notes:
#  - Pure fp16 matmul streamer: log(q) is precomputed on the host (the
#    reference notes enqueue() precomputes log terms offline), removing all
#    ACT/DVE work; the self term rides on the host in fp64.
#  - The a shard is laid out chunk-major in HBM: each chunk (4 d-tiles x
#    2048 anchors) is one contiguous 2 MB block whose 128 per-partition runs
#    (16 KB) sit back-to-back, so the DMA reads one sequential stream with
#    large descriptors.
#  - qlog is loaded once per pass into a resident SBUF tile (12.7 KB/
#    partition, double-buffered) and reused as matmul weights for all 99
#    d-tiles.
#  - a-chunk DMAs are split across the two HWDGE rings (SP=nc.sync,
#    ACT=nc.scalar) so both rings move comparable bytes per pass.
#  - fp16 streams: klD error vs fp64 is ~0.05 rms against a 0.20 min top8/9
#    margin for this data; predictions match the fp32 reference exactly.
#    (fp8 streams were ruled out: best 1-byte format e3m4 gives 2.4 rms
#    error and flips a prediction; int8-style uniform grids aren't a PE
#    dtype on TRN2.)
#  - PSUM accumulates in fp32 across all 99 d-tiles (4 banks x 2 bufs,
#    alternating per repeat iteration so back-to-back passes overlap).
#  - Measured 144 us HW exec (0/64 mismatches) vs the 195 us baseline
#    (on-device log+self, strided 8KB-run DMAs) and ~160-169 us for the
#    tuned 8-way anchor split; PE (~52 us) and all else hides under the DMA.

import numpy as np

B = 64
K = 4096
DIM = 50257
KNN = 8
EPS = 1e-10
N_CORES = 8
P = 128                          # SBUF partitions / d-tile size
GNT = 99                         # d-tiles per core (4 quarters, zero-padded)
GKL = 2048                       # anchors per core (2 halves)
GCT = 4                          # d-tiles per chunk -> 16KB per-partition runs
D_GRID = 4 * GNT * P             # 50688 (padded; pads contribute exactly 0)
KL = 512                         # PSUM bank row (fp32) / matmul N

# legacy shape names kept for the test harness's null-kernel helper
NT = 393
KL_LOCAL = 512
D_PAD = NT * P

_CACHE = {}


def _gchunks():
    out, t0 = [], 0
    while t0 < GNT:
        ct = min(GCT, GNT - t0)
        out.append((t0, ct))
        t0 += ct
    return out


def _build_nc(repeat=1):
    import concourse.bacc as bacc
    import concourse.tile as tile
    import concourse.mybir as mybir
    from contextlib import nullcontext

    f32 = mybir.dt.float32
    f16 = mybir.dt.float16

    nc = bacc.Bacc("TRN2", target_bir_lowering=False, debug=False,
                   num_devices=N_CORES)
    total = GNT * P * GKL
    aT = nc.dram_tensor("aT", [1, total], f16, kind="ExternalInput")
    qT = nc.dram_tensor("qT", [P, GNT * B], f16, kind="ExternalInput")
    out = nc.dram_tensor("out", [B, GKL], f32, kind="ExternalOutput")

    with tile.TileContext(nc) as tc:
        with (
            tc.tile_pool(name="a_io", bufs=4) as a_io,
            tc.tile_pool(name="q_io", bufs=2) as q_io,
            tc.tile_pool(name="o_sb", bufs=2) as o_sb,
            tc.tile_pool(name="psum", bufs=2, space="PSUM") as psum,
        ):
            loop = tc.For_i(0, repeat, 1) if repeat > 1 else nullcontext()
            with loop:
                q_res = q_io.tile([P, GNT * B], f16, tag="q")
                nc.scalar.dma_start(q_res[:], qT.ap()[:, :])

                cross_ps = psum.tile([B, GKL], f32, tag="ps")

                off = 0
                for ci, (t0, ct) in enumerate(_gchunks()):
                    a_tile = a_io.tile([P, GCT * GKL], f16, tag="a")
                    # alternate a-chunks across the rings: scalar gets 12 of
                    # 25 (+ the 1.6MB q load), sync 13 (+ the out write), so
                    # both HWDGE rings move ~27MB per pass
                    eng = nc.scalar if ci % 2 == 1 else nc.sync
                    eng.dma_start(a_tile[:, :ct * GKL],
                                  aT.ap()[:, off:off + ct * P * GKL])
                    off += ct * P * GKL

                    for i in range(ct):
                        t = t0 + i
                        for blk in range(GKL // KL):
                            nc.tensor.matmul(
                                cross_ps[:, blk * KL:(blk + 1) * KL],
                                q_res[:, t * B:(t + 1) * B],
                                a_tile[:, i * GKL + blk * KL:
                                       i * GKL + (blk + 1) * KL],
                                start=(t == 0), stop=(t == GNT - 1))

                out_sb = o_sb.tile([B, GKL], f32, tag="o")
                nc.vector.tensor_copy(out_sb[:], cross_ps[:])
                nc.sync.dma_start(out[:], out_sb[:])

    nc.compile()
    return nc


def get_nc():
    if "nc" not in _CACHE:
        _CACHE["nc"] = _build_nc()
    return _CACHE["nc"]


def compute_self(queue_anchor):
    """Per-anchor self term in fp64 (enqueue-time precompute), sum units."""
    a = np.asarray(queue_anchor, dtype=np.float64)
    return (a * np.log(a + EPS)).sum(axis=1)         # [K]


def prepare_in_maps(query, queue_anchor):
    """Grid-shard + lay out inputs: per core c = kq*4 + dh, the log(query)
    slice for d-quarter dh (partition-major) and anchor half kq's shard over
    that d-quarter (chunk-major), both fp16."""
    query = np.asarray(query, dtype=np.float32)
    queue_anchor = np.asarray(queue_anchor, dtype=np.float32)
    assert query.shape == (B, DIM) and queue_anchor.shape == (K, DIM)

    qlT = np.zeros((D_GRID, B), dtype=np.float32)
    qlT[:DIM] = np.log(query.astype(np.float64) + EPS).T
    qtiles = qlT.reshape(4 * GNT, P, B)
    aTf = np.zeros((D_GRID, K), dtype=np.float32)
    aTf[:DIM] = queue_anchor.T
    atiles = aTf.reshape(4 * GNT, P, K)

    in_maps = []
    for c in range(N_CORES):
        kq, dh = c // 4, c % 4
        tsl = slice(dh * GNT, (dh + 1) * GNT)
        q16 = np.ascontiguousarray(
            qtiles[tsl].transpose(1, 0, 2).reshape(P, GNT * B)
        ).astype(np.float16)
        at = atiles[tsl, :, kq * GKL:(kq + 1) * GKL]   # [GNT, P, GKL]
        blocks = [at[t0:t0 + ct].transpose(1, 0, 2).reshape(-1)
                  for t0, ct in _gchunks()]
        a16 = np.concatenate(blocks).astype(np.float16)[None, :]
        in_maps.append({"aT": a16, "qT": q16})
    return in_maps


def klD_from_outs(outs, self_sum):
    """outs: list of 8 per-core [64, 2048] fp32 partial cross-sums.
    Returns klD [64, 4096] fp64 (sum units; /D does not change ranking)."""
    klD = np.empty((B, K), dtype=np.float64)
    for kq in range(2):
        cross = sum(np.asarray(outs[kq * 4 + dh], dtype=np.float64)
                    for dh in range(4))
        klD[:, kq * GKL:(kq + 1) * GKL] = (
            self_sum[kq * GKL:(kq + 1) * GKL][None, :] - cross)
    return klD


def postprocess(outs, queue_label, self_sum):
    """Final top-8 + majority vote, matching the reference's jax.lax.top_k /
    argmax tie semantics."""
    lab = np.asarray(queue_label).astype(np.int64)
    klD = klD_from_outs(outs, self_sum)
    # top_k(-kl) takes the 8 largest of -kl (= smallest kl), ties -> lower
    # index; stable ascending argsort matches that.
    top8 = np.argsort(klD, axis=1, kind="stable")[:, :KNN]
    votes1 = lab[top8].sum(axis=1)
    # argmax([count0, count1]) with tie -> 0, so predict 1 iff count1 > 4.
    return (votes1 > KNN // 2).astype(np.int32)


def kernel(query, queue_anchor, queue_label):
    from concourse.bass_utils import run_bass_kernel_spmd

    nc = get_nc()
    in_maps = prepare_in_maps(query, queue_anchor)
    self_sum = compute_self(queue_anchor)
    res = run_bass_kernel_spmd(nc, in_maps, core_ids=list(range(N_CORES)))
    outs = [res.results[c]["out"] for c in range(N_CORES)]
    return postprocess(outs, queue_label, self_sum)
